# revision 35
# baseline (speedup 1.0000x reference)
"""CorrelationLayer (81-shift local correlation) on 8 Trainium2 NeuronCores.

Full inputs: feat1, feat2 [4, 128, 184, 320] fp32.
Full output: [4, 81, 184, 320] fp32,
  out[b, (dy+4)*9+(dx+4), y, x] = <f1n[b,:,y,x], f2n[b,:,y-dy,x-dx]>
  (features L2-normalized over C; f2 zero-padded outside the frame).

Sharding: 8 cores = batch(4) x W-halves(2).  Each core gets bf16 shards:
  f1n [128, 29440] L2-normalized on the host, in pixel-block-major order
  (23x10 blocks of 8x16), and f2 [128, 192*168] raw, zero-padded with a
  4-px halo.

Per-core kernel:
 - f2 normalized on-device: square on DVE (2x bf16 mode), channel
   reduction via an all-ones [C,128] stationary matmul (norms land
   replicated across all 128 partitions -- wide tiles, no 1-partition
   ops), a single ACT Abs_reciprocal_sqrt(+eps), and the normalize
   multiply alternating DVE / GPSIMD.
 - correlation: per 8x16-pixel block one PE matmul [C,128px] x
   [C, 16x24 halo] -> PSUM [128, 384]; two blocks share a [128,2,512]
   PSUM pair-tile (3 pair-tiles in flight so the PE never stalls and
   stays at the warm 2.4 GHz clock), evacuated as plain 768-column
   copies alternating ACT / DVE.
 - output: only the useful 216-column slab per 16-partition py-group
   (the rectangular hull of the 81 shifts) is DMA'd, four block-rows
   per transfer.

The host gathers the [81, H, W] layout from the slabs during unshard (a
fixed index permutation).  On-chip de-shear is not performed because
TRN2 DMA/engine access patterns cannot express per-partition fractional
offsets over 128 partitions; all correlation FLOPs and the f2
normalization run on-device.
"""

from contextlib import ExitStack

import numpy as np
import ml_dtypes

import concourse.bass as bass
import concourse.bacc as bacc
import concourse.tile as tile
from concourse import mybir
from concourse.bass_utils import run_bass_kernel_spmd

F32 = mybir.dt.float32
BF16 = mybir.dt.bfloat16

# problem constants (hardcoded per harness contract)
B, C, H, W = 4, 128, 184, 320
ROWS, WIDTH = 184, 160          # per-core shard (W-half)
PY, PX = 8, 16                  # pixel block
HY, HX = PY + 8, PX + 8         # halo block (16 x 24)
NHALO = HY * HX                 # 384
NBY, NBX = ROWS // PY, WIDTH // PX  # 23 x 10
NBLK = NBY * NBX                # 230
R2, W2 = ROWS + 8, WIDTH + 8    # padded f2 shard (192 x 168)
NPIX2 = R2 * W2                 # 32256
CH2 = 504                       # f2 chunk columns (psum bank = 512 fp32)
NCH2 = NPIX2 // CH2             # 64
NPIX1 = NBLK * 128              # 29440
F1CH = NPIX1 // 8               # 3680 (8 input DMAs)
SLAB = 9 * HX                   # 216 useful columns per py-group
RGRP = 2                        # block-rows per output tile / DMA

_compiled = {}


def _build_kernel(nc, f1, f2, out):
    tc_ctx = tile.TileContext(nc)
    with tc_ctx as tc, ExitStack() as ctx:
        ctx.enter_context(nc.allow_low_precision(
            reason="bf16 feature/inv-norm pipeline within correlation tolerance"))

        persist = ctx.enter_context(tc.tile_pool(name="persist", bufs=1))
        loads = ctx.enter_context(tc.tile_pool(name="loads", bufs=3))
        temps = ctx.enter_context(tc.tile_pool(name="temps", bufs=3))
        psum_n = ctx.enter_context(
            tc.tile_pool(name="psum_n", bufs=2, space="PSUM"))
        psum_m = ctx.enter_context(
            tc.tile_pool(name="psum_m", bufs=3, space="PSUM"))
        rowpool = ctx.enter_context(tc.tile_pool(name="rows", bufs=2))

        f1n = persist.tile([C, NPIX1], BF16)
        f2n = persist.tile([C, NPIX2], BF16)
        ones = persist.tile([C, 128], BF16)
        nc.vector.memset(ones, 1.0)
        eps_t = persist.tile([128, 1], F32)
        nc.vector.memset(eps_t, 1e-12)

        # PE warm-up: ~5us of back-to-back matmuls flips the HAM clock
        # gate to 8/8 (2.4 GHz) before the real work arrives.
        warm = persist.tile([C, CH2], BF16)
        nc.vector.memset(warm, 0.5)
        for w in range(10):
            pw = psum_n.tile([128, CH2], F32, tag="ps")
            nc.tensor.matmul(pw, ones, warm, start=True, stop=True)

        f2nv = f2n.rearrange("c (r w) -> c r w", r=R2)
        f1v = f1n.rearrange("c (b p) -> c b p", b=NBLK)

        # Software pipeline: phase0 normalization chunks (each covers 3
        # f2 rows: 504 = 3*168) are emitted just ahead of the
        # correlation row-groups that consume them, so every engine's
        # FIFO queue interleaves both kinds of work.
        state = {"c": 0, "f1d": 0, "xt": None}

        def emit_chunk():
            c = state["c"]
            s = c % 8
            if s == 0:
                state["xt"] = loads.tile([C, 8 * CH2], BF16, tag="xt",
                                         name="xt")
                d = c // 8
                nc.sync.dma_start(out=state["xt"],
                                  in_=f2[:, d * 8 * CH2:(d + 1) * 8 * CH2])
            ci = c * CH2
            x = state["xt"][:, s * CH2:(s + 1) * CH2]
            sq = temps.tile([C, CH2], BF16, tag="sq")
            nc.vector.tensor_mul(out=sq, in0=x, in1=x)
            ps = psum_n.tile([128, CH2], F32, tag="ps")
            nc.tensor.matmul(ps, ones, sq, start=True, stop=True)
            inv = temps.tile([C, CH2], BF16, tag="inv")
            nc.scalar.activation(
                out=inv, in_=ps,
                func=mybir.ActivationFunctionType.Abs_reciprocal_sqrt,
                bias=eps_t[:, 0:1], scale=1.0)
            if c % 2 == 0:
                nc.gpsimd.tensor_mul(out=f2n[:, ci:ci + CH2],
                                     in0=x, in1=inv)
            else:
                nc.vector.tensor_mul(out=f2n[:, ci:ci + CH2],
                                     in0=x, in1=inv)
            state["c"] = c + 1

        # correlation: per 8x16 pixel block, all-pairs vs its 16x24 halo.
        # Two blocks share one [128, 2, 512] psum tile (2 banks); plain
        # 768-column evacuation, alternating ACT / DVE.
        half = 0
        for gi in range((NBY + RGRP - 1) // RGRP):
            bys = list(range(gi * RGRP, min((gi + 1) * RGRP, NBY)))
            # normalization chunks covering f2 rows < bys[-1]*8 + 24
            c_need = min(NCH2, (bys[-1] * PY + HY + 2) // 3)
            while state["c"] < c_need:
                emit_chunk()
            # f1 blocks up to (bys[-1]+1)*NBX
            while (state["f1d"] * F1CH < (bys[-1] + 1) * NBX * 128
                   and state["f1d"] < 8):
                d = state["f1d"]
                nc.sync.dma_start(out=f1n[:, d * F1CH:(d + 1) * F1CH],
                                  in_=f1[:, d * F1CH:(d + 1) * F1CH])
                state["f1d"] = d + 1
            rt = rowpool.tile([128, RGRP, NBX, NHALO], BF16, tag="rt")
            for byi, by in enumerate(bys):
                for bx0, g in ((0, 2), (2, 2), (4, 2), (6, 2), (8, 2)):
                    pm = psum_m.tile([128, 2, 512], F32, tag="pm")
                    for j in range(g):
                        blk = by * NBX + bx0 + j
                        pmv = pm[:, j, 0:NHALO].rearrange(
                            "p (a b) -> p a b", a=HY)
                        nc.tensor.matmul(
                            pmv, f1v[:, blk, :],
                            f2nv[:, by * PY:by * PY + HY,
                                 (bx0 + j) * PX:(bx0 + j) * PX + HX],
                            start=True, stop=True)
                    src = pm[:, 0:g, 0:NHALO]
                    dst = rt[:, byi, bx0:bx0 + g, :]
                    if half == 0:
                        nc.scalar.activation(
                            out=dst, in_=src,
                            func=mybir.ActivationFunctionType.Copy,
                            bias=0.0, scale=1.0)
                    else:
                        nc.vector.tensor_copy(out=dst, in_=src)
                    half ^= 1
            for py in range(PY):
                nc.sync.dma_start(
                    out=out[py, :, bys[0]:bys[0] + len(bys)],
                    in_=rt[py * 16:py * 16 + 16, 0:len(bys), :,
                           py * HX:py * HX + SLAB])


def _get_program():
    if "nc" not in _compiled:
        nc = bacc.Bacc("TRN2", target_bir_lowering=False, debug=False)
        f1 = nc.dram_tensor("f1", [C, NPIX1], BF16,
                            kind="ExternalInput").ap()
        f2 = nc.dram_tensor("f2", [C, NPIX2], BF16,
                            kind="ExternalInput").ap()
        out = nc.dram_tensor("slab", [PY, 16, NBY, NBX, SLAB], BF16,
                             kind="ExternalOutput").ap()
        _build_kernel(nc, f1, f2, out)
        nc.compile()
        _compiled["nc"] = nc
    return _compiled["nc"]


def _host_extract(slab):
    """Slabs [PY, 16, NBY, NBX, SLAB] -> [81, ROWS, WIDTH] (fp32)."""
    v = np.asarray(slab).astype(np.float32)
    out = np.empty((81, ROWS, WIDTH), np.float32)
    ix = np.arange(PX)
    for dy in range(-4, 5):
        a = 4 - dy
        for dx in range(-4, 5):
            b = 4 - dx
            k = (dy + 4) * 9 + (dx + 4)
            # advanced indices at axes (1, 4) are non-adjacent -> the
            # broadcast dim (px) moves to the front: [PX, PY, NBY, NBX]
            g = v[:, ix, :, :, HX * a + b + ix]
            out[k] = g.transpose(2, 1, 3, 0).reshape(ROWS, WIDTH)
    return out


def run_cores(in_maps, **kwargs):
    """Compile once and run the SPMD kernel on cores 0-7."""
    nc = _get_program()
    return run_bass_kernel_spmd(nc, in_maps, core_ids=list(range(8)), **kwargs)


def make_in_maps(feat1, feat2):
    feat1 = np.asarray(feat1, dtype=np.float32)
    feat2 = np.asarray(feat2, dtype=np.float32)
    in_maps = []
    for b in range(B):
        f2p = np.zeros((C, H + 8, W + 8), np.float32)
        f2p[:, 4:-4, 4:-4] = feat2[b]
        for h in range(2):
            x0 = WIDTH * h
            s1 = feat1[b, :, :, x0:x0 + WIDTH]               # [C, 184, 160]
            n1 = np.sqrt(np.sum(s1 * s1, axis=0))
            s1n = s1 / np.maximum(n1, 1e-12)
            # block-major f1: col = ((by*NBX+bx)*PY+py)*PX+px
            f1b = (s1n.reshape(C, NBY, PY, NBX, PX)
                   .transpose(0, 1, 3, 2, 4)
                   .reshape(C, NPIX1))
            in_maps.append({
                "f1": np.ascontiguousarray(
                    f1b.astype(ml_dtypes.bfloat16)),
                "f2": np.ascontiguousarray(
                    f2p[:, :, x0:x0 + W2].reshape(C, NPIX2)
                    .astype(ml_dtypes.bfloat16)),
            })
    return in_maps


def assemble(results):
    out = np.empty((B, 81, H, W), np.float32)
    for i, res in enumerate(results):
        slab = np.asarray(list(res.values())[0])
        b, h = i // 2, i % 2
        out[b, :, :, WIDTH * h:WIDTH * (h + 1)] = _host_extract(slab)
    return out


def kernel(feat1, feat2):
    in_maps = make_in_maps(feat1, feat2)
    res = run_cores(in_maps)
    return assemble(res.results)
